# revision 22
# baseline (speedup 1.0000x reference)
"""Trainium2 Bass kernel for nn_CorrelationLayer (441-displacement cost volume).

result[k, i, j] = sum_c f1[c, i, j] * pad(f2)[c, i + dy_k, j + dx_k]
with (dy, dx) in {0, 2, ..., 40}^2, H, W = 48, 64, C = 128, pad D = 20.

Strategy
--------
All displacements are even, so the problem decomposes into 4 independent
parity sub-problems: for (pi, pj) in {0,1}^2, the slices
f1[:, pi::2, pj::2] and f2[:, pi::2, pj::2] (both [C, 24, 32]) interact
with a dense +-10 displacement window in the halved coordinates:

    out[dy, dx, i, j] = M[r2p, jj2, ii, jj],  r2p = ii + dy - 10,
                                              jj2 = jj + dx - 10,
with ii = i>>1, jj = j>>1.  Each of the 8 cores takes one (sub-problem,
row-half) pair: stationary operand = 12 f2s rows packed as 3 tiles of
[C, 4 rows x 32 cols = 128], moving operand = f1s rows [C, 24*32 = 768].
Only the moving prefix that can fall inside the +-10 row band is
computed (ii <= 4t + 13 for tile t; the upper row-half cores flip the
row axis on the host so the same program serves all 8 cores).  The band
gather, zero padding, and row un-flip are pure data rearrangement done
on the host during unsharding -- all arithmetic happens on device.
"""

import sys
import types

for _p in ("/opt/trn_rl_repo", "/root/.axon_site"):
    if _p not in sys.path:
        sys.path.insert(0, _p)

import ml_dtypes
import numpy as np

BF16 = ml_dtypes.bfloat16

import concourse.bacc as bacc
import concourse.mybir as mybir
from concourse import tile
from concourse import bass_utils
from concourse.bass_utils import run_bass_kernel_spmd

C = 128
H = 48
W = 64
ND = 21          # displacements per axis
NCORES = 8
HH = H // 2      # 24 rows per parity slice
HW = W // 2      # 32 cols per parity slice
RPC = HH // 2    # 12 stationary f2s rows per core
F2COLS = RPC * HW        # 384 stationary columns
F1COLS = HH * HW         # 768 moving columns
INCOLS = F2COLS + F1COLS  # 1152 packed input columns
# moving-column windows per stationary tile t (rows 4t..4t+3 need
# ii <= 4t + 3 + 10)
NT = [14 * HW, 18 * HW, 22 * HW]       # 448, 576, 704
OUTCOLS = sum(NT)                       # 1728
OFFS = [0, NT[0], NT[0] + NT[1]]        # 0, 448, 1024
F1USED = NT[2]                          # moving cols 704:768 are never read
INCOLS_USED = F2COLS + F1USED           # 1088 packed input columns


def _ensure_ntff_hook():
    """Register the axon NTFF profile hook if possible (for trace runs)."""
    try:
        import antenv
        if "antenv.axon_hooks" not in sys.modules:
            mod = types.ModuleType("antenv.axon_hooks")
            _h = [None]
            mod.set_axon_ntff_profile_hook = lambda h: _h.__setitem__(0, h)
            mod.get_axon_ntff_profile_hook = lambda: _h[0]
            sys.modules["antenv.axon_hooks"] = mod
            antenv.axon_hooks = mod
        bass_utils.upload_artifacts = lambda tmpdir: "local://" + tmpdir
        from trn_agent_boot.trn_boot import _ntff_profile_via_ctypes
        sys.modules["antenv.axon_hooks"].set_axon_ntff_profile_hook(
            _ntff_profile_via_ctypes("/opt/axon/libaxon_pjrt.so")
        )
    except Exception:
        pass


def build_program():
    nc = bacc.Bacc(None, target_bir_lowering=False)
    inp = nc.declare_dram_parameter(
        "inp", [C, INCOLS_USED], mybir.dt.bfloat16, isOutput=False
    )
    mout = nc.declare_dram_parameter("mout", [C, OUTCOLS], mybir.dt.bfloat16, isOutput=True)

    with tile.TileContext(nc) as tc:
        with (
            tc.tile_pool(name="in", bufs=1) as in_pool,
            tc.tile_pool(name="out", bufs=1) as out_pool,
            tc.tile_pool(name="ps", bufs=1, space="PSUM") as ps_pool,
        ):
            inp_sb = in_pool.tile([C, INCOLS_USED], mybir.dt.bfloat16)
            # one input DMA: descriptor count is per-partition (128) no
            # matter the width, so splitting doubles descriptor work; sync
            # (SP) has the shortest DGE trigger latency (650ns)
            nc.sync.dma_start(out=inp_sb[:], in_=inp[:])

            # The HAM clock manager reaches max frequency (PE 2.4 GHz, and a
            # ~1.4x chip-wide boost) only after ~3 us of CONTINUOUS
            # full-width PE activity.  Dependency-free 512-row warmup matmuls
            # keep the PE busy from program start until the input lands,
            # overlapping the input DMA (narrow or K=1 warmups do NOT
            # trigger the ramp).
            scr = in_pool.tile([C, 512], mybir.dt.bfloat16, tag="scr")
            nc.gpsimd.memset(scr[:], 0)
            ps_warm = ps_pool.tile([C, 512], mybir.dt.float32, tag="psw")
            for _ in range(5):
                nc.tensor.matmul(
                    ps_warm[:, :512],
                    scr[:, :128],
                    scr[:, :512],
                    start=True,
                    stop=True,
                )

            big = out_pool.tile([C, OUTCOLS], mybir.dt.bfloat16)
            ps0 = ps_pool.tile([C, NT[0]], mybir.dt.float32, tag="ps0")
            ps1 = ps_pool.tile([C, NT[1]], mybir.dt.float32, tag="ps1")
            ps2 = ps_pool.tile([C, NT[2]], mybir.dt.float32, tag="ps2")
            pss = [ps0, ps1, ps2]
            F = F2COLS
            # big tiles (t1, t2) first so their copies overlap later matmuls
            # and the final copy->issue->transfer chain carries only t0
            # (448 cols)
            def mm(t, c0, c1):
                nc.tensor.matmul(
                    pss[t][:, c0:c1],
                    inp_sb[:, t * 128 : (t + 1) * 128],
                    inp_sb[:, F + c0 : F + c1],
                    start=True,
                    stop=True,
                )

            mm(1, 0, 512)
            mm(2, 0, 512)
            mm(1, 512, NT[1])
            mm(2, 512, NT[2])
            mm(0, 0, NT[0])
            # PSUM -> SBUF (cast to bf16) split over DVE and ACT; out-DMAs on
            # the sync (SP) queue (650ns trigger latency vs 784 on ACT)
            nc.vector.tensor_copy(big[:, OFFS[2] : OFFS[2] + NT[2]], pss[2][:])
            nc.sync.dma_start(
                out=mout[:, OFFS[2] : OFFS[2] + NT[2]],
                in_=big[:, OFFS[2] : OFFS[2] + NT[2]],
            )
            nc.scalar.copy(big[:, OFFS[1] : OFFS[1] + NT[1]], pss[1][:])
            nc.scalar.dma_start(
                out=mout[:, OFFS[1] : OFFS[1] + NT[1]],
                in_=big[:, OFFS[1] : OFFS[1] + NT[1]],
            )
            nc.vector.tensor_copy(big[:, OFFS[0] : OFFS[0] + NT[0]], pss[0][:])
            nc.sync.dma_start(
                out=mout[:, OFFS[0] : OFFS[0] + NT[0]],
                in_=big[:, OFFS[0] : OFFS[0] + NT[0]],
            )
    nc.compile()
    return nc


_PROGRAM_CACHE = {}


def _get_program():
    if "nc" not in _PROGRAM_CACHE:
        _PROGRAM_CACHE["nc"] = build_program()
    return _PROGRAM_CACHE["nc"]


def _shard_inputs(features_1, features_2):
    """Per-core input maps. Core m: sub-problem (pi, pj) = (m>>1 & 1, m & 1),
    row-half m>>2 (upper half cores flip the row axis so one program fits
    all).  Packed input = [f2s 12 rows | f1s 22 rows] = [128, 1088] bf16
    (f1s rows 22-23 can never fall in the +-10 band of this core's f2s
    rows, so they are not shipped)."""
    f1 = np.asarray(features_1, dtype=np.float32)
    f2 = np.asarray(features_2, dtype=np.float32)
    in_maps = []
    for m in range(NCORES):
        pi, pj, half = (m >> 1) & 1, m & 1, m >> 2
        f1s = f1[:, pi::2, pj::2]
        f2s = f2[:, pi::2, pj::2]
        if half:
            f1s = f1s[:, ::-1, :]
            f2s = f2s[:, ::-1, :]
        packed = np.empty((C, INCOLS_USED), dtype=BF16)
        packed[:, :F2COLS] = f2s[:, :RPC, :].reshape(C, F2COLS)
        packed[:, F2COLS:] = f1s[:, : F1USED // HW, :].reshape(C, F1USED)
        in_maps.append({"inp": packed})
    return in_maps


def _assemble(results):
    """Gather the 2D displacement band from the per-core all-pairs tiles."""
    # Msub[pi, pj, r2p, jj2, ii, jj] = sum_c f2s[c, r2p, jj2] * f1s[c, ii, jj]
    Msub = np.empty((2, 2, HH, HW, HH, HW), dtype=np.float32)
    for m in range(NCORES):
        pi, pj, half = (m >> 1) & 1, m & 1, m >> 2
        raw = np.asarray(results[m]["mout"]).astype(np.float32)
        Mc = np.empty((RPC, HW, HH, HW), dtype=np.float32)
        for t in range(3):
            nii = NT[t] // HW
            blk = raw[:, OFFS[t] : OFFS[t] + NT[t]].reshape(4, HW, nii, HW)
            Mc[4 * t : 4 * t + 4, :, :nii, :] = blk
        if half:
            # core computed flipped rows: M'[r2p', jj2, ii', jj] with
            # r2p' = 23 - r2p, ii' = 23 - ii
            Msub[pi, pj, RPC:] = Mc[::-1, :, ::-1, :]
        else:
            Msub[pi, pj, :RPC] = Mc
    dy, dxi, i, j = np.ogrid[0:ND, 0:ND, 0:H, 0:W]
    ii, jj = i >> 1, j >> 1
    r2p = ii + dy - (ND // 2)
    jj2 = jj + dxi - (ND // 2)
    valid = (r2p >= 0) & (r2p < HH) & (jj2 >= 0) & (jj2 < HW)
    r2c = np.clip(r2p, 0, HH - 1)
    jjc = np.clip(jj2, 0, HW - 1)
    pi = (i & 1) + np.zeros_like(r2c)
    pj = (j & 1) + np.zeros_like(r2c)
    out = Msub[pi, pj, r2c, jjc, ii + np.zeros_like(r2c), jj + np.zeros_like(r2c)]
    out[~valid] = 0.0
    return out.reshape(1, ND * ND, H, W)


def kernel(features_1, features_2):
    nc = _get_program()
    in_maps = _shard_inputs(features_1, features_2)
    res = run_bass_kernel_spmd(nc, in_maps, list(range(NCORES)))
    return _assemble(res.results)


def kernel_traced(features_1, features_2, tmpdir=None):
    """Same as kernel() but with NTFF profiling; returns (output, exec_time_ns)."""
    _ensure_ntff_hook()
    nc = _get_program()
    in_maps = _shard_inputs(features_1, features_2)
    res = run_bass_kernel_spmd(
        nc, in_maps, list(range(NCORES)), trace=True, tmpdir=tmpdir
    )
    return _assemble(res.results), res.exec_time_ns


# revision 26
# speedup vs baseline: 1.0574x; 1.0574x over previous
"""Trainium2 Bass kernel for nn_CorrelationLayer (441-displacement cost volume).

result[k, i, j] = sum_c f1[c, i, j] * pad(f2)[c, i + dy_k, j + dx_k]
with (dy, dx) in {0, 2, ..., 40}^2, H, W = 48, 64, C = 128, pad D = 20.

Strategy
--------
All displacements are even, so the problem decomposes into 4 independent
parity sub-problems: for (pi, pj) in {0,1}^2, the slices
f1[:, pi::2, pj::2] and f2[:, pi::2, pj::2] (both [C, 24, 32]) interact
with a dense +-10 displacement window in the halved coordinates:

    out[dy, dx, i, j] = M[r2p, jj2, ii, jj],  r2p = ii + dy - 10,
                                              jj2 = jj + dx - 10,
with ii = i>>1, jj = j>>1.  Each of the 8 cores takes one (sub-problem,
row-half) pair: stationary operand = 12 f2s rows packed as 3 tiles of
[C, 4 rows x 32 cols = 128], moving operand = f1s rows [C, 24*32 = 768].
Only the moving prefix that can fall inside the +-10 row band is
computed (ii <= 4t + 13 for tile t; the upper row-half cores flip the
row axis on the host so the same program serves all 8 cores).  The band
gather, zero padding, and row un-flip are pure data rearrangement done
on the host during unsharding -- all arithmetic happens on device.
"""

import sys
import types

for _p in ("/opt/trn_rl_repo", "/root/.axon_site"):
    if _p not in sys.path:
        sys.path.insert(0, _p)

import ml_dtypes
import numpy as np

BF16 = ml_dtypes.bfloat16

import concourse.bacc as bacc
import concourse.mybir as mybir
from concourse import tile
from concourse import bass_utils
from concourse.bass_utils import run_bass_kernel_spmd

C = 128
H = 48
W = 64
ND = 21          # displacements per axis
NCORES = 8
HH = H // 2      # 24 rows per parity slice
HW = W // 2      # 32 cols per parity slice
RPC = HH // 2    # 12 stationary f2s rows per core
F2COLS = RPC * HW        # 384 stationary columns
F1COLS = HH * HW         # 768 moving columns
INCOLS = F2COLS + F1COLS  # 1152 packed input columns
# moving-column windows per stationary tile t (rows 4t..4t+3 need
# ii <= 4t + 3 + 10)
NT = [14 * HW, 18 * HW, 22 * HW]       # 448, 576, 704
OUTCOLS = sum(NT)                       # 1728
OFFS = [0, NT[0], NT[0] + NT[1]]        # 0, 448, 1024
F1USED = NT[2]                          # moving cols 704:768 are never read
INCOLS_USED = F2COLS + F1USED           # 1088 packed input columns


def _ensure_ntff_hook():
    """Register the axon NTFF profile hook if possible (for trace runs)."""
    try:
        import antenv
        if "antenv.axon_hooks" not in sys.modules:
            mod = types.ModuleType("antenv.axon_hooks")
            _h = [None]
            mod.set_axon_ntff_profile_hook = lambda h: _h.__setitem__(0, h)
            mod.get_axon_ntff_profile_hook = lambda: _h[0]
            sys.modules["antenv.axon_hooks"] = mod
            antenv.axon_hooks = mod
        bass_utils.upload_artifacts = lambda tmpdir: "local://" + tmpdir
        from trn_agent_boot.trn_boot import _ntff_profile_via_ctypes
        sys.modules["antenv.axon_hooks"].set_axon_ntff_profile_hook(
            _ntff_profile_via_ctypes("/opt/axon/libaxon_pjrt.so")
        )
    except Exception:
        pass


def build_program():
    nc = bacc.Bacc(None, target_bir_lowering=False)
    inp = nc.declare_dram_parameter(
        "inp", [C, INCOLS_USED], mybir.dt.bfloat16, isOutput=False
    )
    mout = nc.declare_dram_parameter("mout", [C, OUTCOLS], mybir.dt.bfloat16, isOutput=True)

    with tile.TileContext(nc) as tc:
        with (
            tc.tile_pool(name="in", bufs=1) as in_pool,
            tc.tile_pool(name="out", bufs=1) as out_pool,
            tc.tile_pool(name="ps", bufs=1, space="PSUM") as ps_pool,
        ):
            inp_sb = in_pool.tile([C, INCOLS_USED], mybir.dt.bfloat16)
            # input in two DMAs, BOTH on sync (SP, shortest 650ns DGE
            # trigger latency) so their descriptors drain in order: A
            # carries f2 + moving[0:512], which is all the first two matmul
            # chunks need; B lands while those chunks execute
            SPL = F2COLS + 512
            nc.sync.dma_start(out=inp_sb[:, :SPL], in_=inp[:, :SPL])
            nc.sync.dma_start(out=inp_sb[:, SPL:], in_=inp[:, SPL:])

            # The HAM clock manager reaches max frequency (PE 2.4 GHz, and a
            # ~1.4x chip-wide boost) only after ~3 us of CONTINUOUS
            # full-width PE activity.  Dependency-free 512-row warmup matmuls
            # keep the PE busy from program start until the input lands,
            # overlapping the input DMA (narrow or K=1 warmups do NOT
            # trigger the ramp).
            # warmup operands don't need real values -- memset just the
            # minimum the tile allocator requires so the first warmup can
            # start (and the HAM ramp timer) ~0.5us earlier
            scr = in_pool.tile([C, 512], mybir.dt.bfloat16, tag="scr")
            nc.gpsimd.memset(scr[:, :8], 0)
            ps_warm = ps_pool.tile([C, 512], mybir.dt.float32, tag="psw")
            for _ in range(6):
                nc.tensor.matmul(
                    ps_warm[:, :512],
                    scr[:, :128],
                    scr[:, :512],
                    start=True,
                    stop=True,
                )

            big = out_pool.tile([C, OUTCOLS], mybir.dt.bfloat16)
            ps0 = ps_pool.tile([C, NT[0]], mybir.dt.float32, tag="ps0")
            ps1 = ps_pool.tile([C, NT[1]], mybir.dt.float32, tag="ps1")
            ps2 = ps_pool.tile([C, NT[2]], mybir.dt.float32, tag="ps2")
            pss = [ps0, ps1, ps2]
            F = F2COLS
            # big tiles (t1, t2) first so their copies overlap later matmuls
            # and the final copy->issue->transfer chain carries only t0
            # (448 cols)
            def mm(t, c0, c1):
                nc.tensor.matmul(
                    pss[t][:, c0:c1],
                    inp_sb[:, t * 128 : (t + 1) * 128],
                    inp_sb[:, F + c0 : F + c1],
                    start=True,
                    stop=True,
                )

            mm(1, 0, 512)
            mm(2, 0, 512)
            mm(1, 512, NT[1])
            mm(2, 512, NT[2])
            mm(0, 0, NT[0])
            # PSUM -> SBUF (cast to bf16) split over DVE and ACT; out-DMAs on
            # the sync (SP) queue (650ns trigger latency vs 784 on ACT)
            nc.vector.tensor_copy(big[:, OFFS[2] : OFFS[2] + NT[2]], pss[2][:])
            nc.sync.dma_start(
                out=mout[:, OFFS[2] : OFFS[2] + NT[2]],
                in_=big[:, OFFS[2] : OFFS[2] + NT[2]],
            )
            nc.scalar.copy(big[:, OFFS[1] : OFFS[1] + NT[1]], pss[1][:])
            nc.scalar.dma_start(
                out=mout[:, OFFS[1] : OFFS[1] + NT[1]],
                in_=big[:, OFFS[1] : OFFS[1] + NT[1]],
            )
            nc.vector.tensor_copy(big[:, OFFS[0] : OFFS[0] + NT[0]], pss[0][:])
            nc.sync.dma_start(
                out=mout[:, OFFS[0] : OFFS[0] + NT[0]],
                in_=big[:, OFFS[0] : OFFS[0] + NT[0]],
            )
    nc.compile()
    return nc


_PROGRAM_CACHE = {}


def _get_program():
    if "nc" not in _PROGRAM_CACHE:
        _PROGRAM_CACHE["nc"] = build_program()
    return _PROGRAM_CACHE["nc"]


def _shard_inputs(features_1, features_2):
    """Per-core input maps. Core m: sub-problem (pi, pj) = (m>>1 & 1, m & 1),
    row-half m>>2 (upper half cores flip the row axis so one program fits
    all).  Packed input = [f2s 12 rows | f1s 22 rows] = [128, 1088] bf16
    (f1s rows 22-23 can never fall in the +-10 band of this core's f2s
    rows, so they are not shipped)."""
    f1 = np.asarray(features_1, dtype=np.float32)
    f2 = np.asarray(features_2, dtype=np.float32)
    in_maps = []
    for m in range(NCORES):
        pi, pj, half = (m >> 1) & 1, m & 1, m >> 2
        f1s = f1[:, pi::2, pj::2]
        f2s = f2[:, pi::2, pj::2]
        if half:
            f1s = f1s[:, ::-1, :]
            f2s = f2s[:, ::-1, :]
        packed = np.empty((C, INCOLS_USED), dtype=BF16)
        packed[:, :F2COLS] = f2s[:, :RPC, :].reshape(C, F2COLS)
        packed[:, F2COLS:] = f1s[:, : F1USED // HW, :].reshape(C, F1USED)
        in_maps.append({"inp": packed})
    return in_maps


def _assemble(results):
    """Gather the 2D displacement band from the per-core all-pairs tiles."""
    # Msub[pi, pj, r2p, jj2, ii, jj] = sum_c f2s[c, r2p, jj2] * f1s[c, ii, jj]
    Msub = np.empty((2, 2, HH, HW, HH, HW), dtype=np.float32)
    for m in range(NCORES):
        pi, pj, half = (m >> 1) & 1, m & 1, m >> 2
        raw = np.asarray(results[m]["mout"]).astype(np.float32)
        Mc = np.empty((RPC, HW, HH, HW), dtype=np.float32)
        for t in range(3):
            nii = NT[t] // HW
            blk = raw[:, OFFS[t] : OFFS[t] + NT[t]].reshape(4, HW, nii, HW)
            Mc[4 * t : 4 * t + 4, :, :nii, :] = blk
        if half:
            # core computed flipped rows: M'[r2p', jj2, ii', jj] with
            # r2p' = 23 - r2p, ii' = 23 - ii
            Msub[pi, pj, RPC:] = Mc[::-1, :, ::-1, :]
        else:
            Msub[pi, pj, :RPC] = Mc
    dy, dxi, i, j = np.ogrid[0:ND, 0:ND, 0:H, 0:W]
    ii, jj = i >> 1, j >> 1
    r2p = ii + dy - (ND // 2)
    jj2 = jj + dxi - (ND // 2)
    valid = (r2p >= 0) & (r2p < HH) & (jj2 >= 0) & (jj2 < HW)
    r2c = np.clip(r2p, 0, HH - 1)
    jjc = np.clip(jj2, 0, HW - 1)
    pi = (i & 1) + np.zeros_like(r2c)
    pj = (j & 1) + np.zeros_like(r2c)
    out = Msub[pi, pj, r2c, jjc, ii + np.zeros_like(r2c), jj + np.zeros_like(r2c)]
    out[~valid] = 0.0
    return out.reshape(1, ND * ND, H, W)


def kernel(features_1, features_2):
    nc = _get_program()
    in_maps = _shard_inputs(features_1, features_2)
    res = run_bass_kernel_spmd(nc, in_maps, list(range(NCORES)))
    return _assemble(res.results)


def kernel_traced(features_1, features_2, tmpdir=None):
    """Same as kernel() but with NTFF profiling; returns (output, exec_time_ns)."""
    _ensure_ntff_hook()
    nc = _get_program()
    in_maps = _shard_inputs(features_1, features_2)
    res = run_bass_kernel_spmd(
        nc, in_maps, list(range(NCORES)), trace=True, tmpdir=tmpdir
    )
    return _assemble(res.results), res.exec_time_ns
